# revision 8
# baseline (speedup 1.0000x reference)
"""DifferentiableLengthRegulator Trainium2 kernel.

out[b,c,l] = y_mask * (sum_t x[b,c,t]*W[b,t,l]) / (sum_t W[b,t,l] + eps)
W = exp(-0.5*(l - center[b,t])^2 / (w[b,t]^2*sigma_scale^2 + eps))

Sharding: data-parallel over batch B=16 -> 8 cores x 2 batches.
Per core, per batch (banded over the frame axis, since the Gaussian
weights vanish outside ~13 sigma of each token chunk's centers):
  DVE : mu = pos - c                      (tensor_scalar, 2x fp32)
  POOL: E  = (mu*alpha)*mu                (scalar_tensor_tensor)
  ACT : W  = exp(E) -> bf16
  PE  : psum[l,0:257] = sum_tc W_tc[:,l-slice]^T @ [xT | ones]  (bf16)
  DVE : rd = y_mask/(psum[:,256]+eps)
  ACT/DVE: out_sb[l,c] = psum[l,0:256] * rd   (PSUM->SBUF move fused)
Output written (B, L, C)-contiguous; host returns the transpose view.
"""

import numpy as np
import ml_dtypes

B, C, T, L = 16, 256, 512, 4096
N_CORES = 8
BPC = B // N_CORES  # batches per core
CH = 128            # partition chunk
TCN = T // CH       # 4 token chunks
LCN = L // CH       # 32 frame chunks
GRP = 4             # frame chunks per psum group
NGRP = LCN // GRP   # 8 groups
EPS = 1e-8
MARGIN_SIGMA = 13.19

_bf16 = ml_dtypes.bfloat16
_cache = {}


def _center_alpha(w, sigma_scale):
    """Mirror the reference's cumsum/center math (same jax backend bits)."""
    try:
        import jax.numpy as jnp

        wj = jnp.asarray(w)
        center = np.asarray(jnp.cumsum(wj, axis=1) - 0.5 * wj, dtype=np.float32)
    except Exception:
        center = (np.cumsum(w, axis=1, dtype=np.float32) - 0.5 * w).astype(np.float32)
    sigma = (w * np.float32(sigma_scale)).astype(np.float32)
    alpha = (np.float32(-0.5) / (np.square(sigma) + np.float32(EPS))).astype(np.float32)
    return center, alpha


def _bands(center, w_all):
    """Per (slot, tc) 512-aligned frame band, unioned across cores (SPMD)."""
    margin = float(MARGIN_SIGMA * w_all.max() + 1.0)
    bands = []
    for slot in range(BPC):
        rows = center[slot::BPC]  # the 8 batches that land in this slot
        sb = []
        for tc in range(TCN):
            seg = rows[:, tc * CH:(tc + 1) * CH]
            bs = max(0, int(np.floor((seg.min() - margin) / 512)) * 512)
            be = min(L, int(np.ceil((seg.max() + margin) / 512)) * 512)
            if tc == 0:
                bs = 0
            if tc == TCN - 1:
                be = L
            bs = min(bs, be - 512)
            sb.append((bs, be))
        bands.append(sb)
    return bands


def _split_excess_waits(nc, max_waits=1):
    """walrus here caps sync-waits at 2 per instruction; move the excess
    onto injected same-engine NoOps just before the instruction (waiting
    earlier on the same engine is always safe)."""
    from concourse import mybir

    for f in nc.m.functions:
        for blk in f.blocks:
            new = []
            for inst in blk.instructions:
                si = inst.sync_info
                if si is not None and len(si.on_wait) > max_waits:
                    waits = list(si.on_wait)
                    keep, extra = waits[-max_waits:], waits[:-max_waits]
                    for i in range(0, len(extra), max_waits):
                        nop = mybir.InstNoOp(name=f"{inst.name}-xw{i}", ins=[], outs=[])
                        nop.engine = inst.engine
                        nop.sync_info = mybir.SyncInfo(
                            on_wait=extra[i:i + max_waits], on_update=[])
                        new.append(nop)
                    inst.sync_info = mybir.SyncInfo(
                        on_wait=keep, on_update=list(si.on_update))
                new.append(inst)
            blk.instructions = new


def _build(band_key):
    import concourse.bass as bass
    import concourse.tile as tile
    from concourse import mybir

    bands = [[(band_key[s][t][0], band_key[s][t][1]) for t in range(TCN)]
             for s in range(BPC)]
    wmax = max(be - bs for sb in bands for (bs, be) in sb)

    nc = bass.Bass("TRN2", target_bir_lowering=False, debug=False)
    xta_d = nc.declare_dram_parameter("xta", [BPC, T, C + 1], mybir.dt.bfloat16, isOutput=False)
    coefs_d = nc.declare_dram_parameter("coefs", [3 * BPC * TCN, CH], mybir.dt.float32, isOutput=False)
    ym_d = nc.declare_dram_parameter("ym", [BPC * LCN, CH], mybir.dt.float32, isOutput=False)
    out_d = nc.declare_dram_parameter("out", [BPC, L, C], mybir.dt.float32, isOutput=True)

    f32 = mybir.dt.float32
    bf16 = mybir.dt.bfloat16
    FT = mybir.ActivationFunctionType
    OP = mybir.AluOpType

    with tile.TileContext(nc) as tc_:
        import contextlib

        with contextlib.ExitStack() as ctx:
            consts = ctx.enter_context(tc_.tile_pool(name="consts", bufs=1))
            xta_p = ctx.enter_context(tc_.tile_pool(name="xta", bufs=2))
            mu_p = ctx.enter_context(tc_.tile_pool(name="mu", bufs=2))
            w_pools = [ctx.enter_context(tc_.tile_pool(name=f"w{t}", bufs=2)) for t in range(TCN)]
            psum_p = ctx.enter_context(tc_.tile_pool(name="ps", bufs=2, space="PSUM"))
            small_p = ctx.enter_context(tc_.tile_pool(name="small", bufs=4))
            out_p = ctx.enter_context(tc_.tile_pool(name="osb", bufs=3))

            # --- constants ---
            pos_i = consts.tile([CH, L], mybir.dt.int32)
            nc.gpsimd.iota(pos_i, pattern=[[1, L]], base=0, channel_multiplier=0)
            pos_f = consts.tile([CH, L], f32)
            nc.vector.tensor_copy(out=pos_f, in_=pos_i)

            coefs_sb = consts.tile([CH, 3 * BPC * TCN], f32)
            nc.sync.dma_start(out=coefs_sb, in_=coefs_d[:, :].rearrange("n p -> p n"))
            ym_sb = consts.tile([CH, BPC * LCN], f32)
            nc.sync.dma_start(out=ym_sb, in_=ym_d[:, :].rearrange("n p -> p n"))

            def col(tile_, idx):
                return tile_[:, idx:idx + 1]

            for b in range(BPC):
                sb = bands[b]
                # index helpers into coefs: param q, batch b, chunk t
                def cidx(q, t):
                    return (q * BPC + b) * TCN + t

                xta_sb = xta_p.tile([CH, TCN, C + 1], bf16)
                nc.sync.dma_start(
                    out=xta_sb,
                    in_=xta_d[b].rearrange("(t p) c -> p t c", p=CH),
                )
                for t in range(TCN):
                    # x_mask fold (per-token scale on the x columns only)
                    nc.vector.tensor_scalar_mul(
                        out=xta_sb[:, t, :C], in0=xta_sb[:, t, :C],
                        scalar1=col(coefs_sb, cidx(2, t)),
                    )

                w_tiles = []
                for t in range(TCN):
                    bs, be = sb[t]
                    bw = be - bs
                    mu = mu_p.tile([CH, wmax], f32)
                    nc.vector.tensor_scalar(
                        out=mu[:, :bw], in0=pos_f[:, bs:be],
                        scalar1=col(coefs_sb, cidx(0, t)), scalar2=None,
                        op0=OP.subtract,
                    )
                    nc.vector.scalar_tensor_tensor(
                        out=mu[:, :bw], in0=mu[:, :bw],
                        scalar=col(coefs_sb, cidx(1, t)), in1=mu[:, :bw],
                        op0=OP.mult, op1=OP.mult,
                    )
                    wt = w_pools[t].tile([CH, wmax], bf16)
                    nc.scalar.activation(out=wt[:, :bw], in_=mu[:, :bw], func=FT.Exp)
                    w_tiles.append(wt)

                for g in range(NGRP):
                    pgrp = psum_p.tile([CH, GRP, 512], f32)
                    for k in range(GRP):
                        j = g * GRP + k
                        lo = j * CH
                        ctc = [t for t in range(TCN) if sb[t][0] <= lo < sb[t][1]]
                        if not ctc:
                            nc.vector.memset(pgrp[:, k, :C + 1], 0.0)
                            continue
                        for i, t in enumerate(ctc):
                            off = lo - sb[t][0]
                            nc.tensor.matmul(
                                out=pgrp[:, k, :C + 1],
                                lhsT=w_tiles[t][:, off:off + CH],
                                rhs=xta_sb[:, t, :],
                                start=(i == 0), stop=(i == len(ctc) - 1),
                            )
                    dtmp = small_p.tile([CH, GRP], f32)
                    nc.vector.tensor_scalar(
                        out=dtmp, in0=pgrp[:, :, C], scalar1=float(EPS),
                        scalar2=None, op0=OP.add,
                    )
                    rd = small_p.tile([CH, GRP], f32)
                    nc.vector.reciprocal(out=rd, in_=dtmp)
                    nc.vector.tensor_mul(
                        out=rd, in0=rd,
                        in1=ym_sb[:, b * LCN + g * GRP: b * LCN + g * GRP + GRP],
                    )
                    ogrp = out_p.tile([CH, GRP, C], f32)
                    for k in range(GRP):
                        if k < 1:
                            nc.vector.tensor_scalar_mul(
                                out=ogrp[:, k, :], in0=pgrp[:, k, :C],
                                scalar1=col(rd, k),
                            )
                        else:
                            nc.scalar.activation(
                                out=ogrp[:, k, :], in_=pgrp[:, k, :C],
                                func=FT.Copy, scale=col(rd, k),
                            )
                    nc.sync.dma_start(
                        out=out_d[b, g * GRP * CH:(g + 1) * GRP * CH, :].rearrange(
                            "(k p) c -> p k c", p=CH),
                        in_=ogrp,
                    )
    _split_excess_waits(nc)
    return nc


def _prepare_inputs(x, w, x_mask, y_mask, sigma_scale):
    center, alpha = _center_alpha(w, sigma_scale[0])
    bands = _bands(center, w)

    xt = np.ascontiguousarray(x.transpose(0, 2, 1))          # (B, T, C)
    xta = np.concatenate([xt, np.ones((B, T, 1), np.float32)], axis=2)
    xta = xta.astype(_bf16)                                   # (B, T, C+1)

    xm = np.broadcast_to(x_mask.reshape(B, T), (B, T)).astype(np.float32)
    ymf = np.broadcast_to(y_mask.reshape(B, L), (B, L)).astype(np.float32)

    in_maps = []
    for core in range(N_CORES):
        bsel = [core * BPC + s for s in range(BPC)]
        coefs = np.empty((3, BPC, TCN, CH), np.float32)
        for s, bb in enumerate(bsel):
            coefs[0, s] = center[bb].reshape(TCN, CH)
            coefs[1, s] = alpha[bb].reshape(TCN, CH)
            coefs[2, s] = xm[bb].reshape(TCN, CH)
        ym_c = np.stack([ymf[bb].reshape(LCN, CH) for bb in bsel])  # (BPC,LCN,CH)
        in_maps.append({
            "xta": xta[bsel],
            "coefs": coefs.reshape(3 * BPC * TCN, CH),
            "ym": ym_c.reshape(BPC * LCN, CH),
        })
    band_key = tuple(tuple(tuple(p) for p in sb) for sb in bands)
    return in_maps, band_key


def kernel(x, w, x_mask, y_mask, sigma_scale):
    x = np.asarray(x, dtype=np.float32)
    w = np.asarray(w, dtype=np.float32)
    x_mask = np.asarray(x_mask, dtype=np.float32)
    y_mask = np.asarray(y_mask, dtype=np.float32)
    sigma_scale = np.asarray(sigma_scale, dtype=np.float32)
    assert x.shape == (B, C, T) and w.shape == (B, T)

    in_maps, band_key = _prepare_inputs(x, w, x_mask, y_mask, sigma_scale)

    if band_key not in _cache:
        _cache[band_key] = _build(band_key)
    nc = _cache[band_key]

    from concourse.bass_utils import run_bass_kernel_spmd

    res = run_bass_kernel_spmd(nc, in_maps, list(range(N_CORES)), trace=False)
    outs = [res.results[i]["out"] for i in range(N_CORES)]      # (BPC, L, C) each
    full = np.concatenate(outs, axis=0)                          # (B, L, C)
    return full.transpose(0, 2, 1)                               # (B, C, L)
